# revision 1
# baseline (speedup 1.0000x reference)
"""DeepClusteringLoss Trainium2 kernel.

loss = (||V^T V||_F^2 - 2 ||V^T E||_F^2 + ||E^T E||_F^2) / (B*N)
summed over batch, with E = embeddings.reshape(B, N, D), V =
assignments.reshape(B, N, S), N = F*T.

Sharding: data-parallel over batch; each of the 8 cores handles one batch
element.  On-core, the combined matrix W = [V | E] (N x 44) is streamed
through the PE array in 1024 chunks of 128 rows, accumulating the full
Gram G = W^T W (44 x 44) in PSUM (fp16 operands, fp32 accumulate).
Blocks of chunks are DMAed contiguously (fp32->fp16 cast in the DMA for
steady blocks), interleaved into per-chunk [V_u | E_u] operands by
DVE/ACT copies, and fed to one matmul per chunk.  The per-core scalar
partial loss = ||G||^2 - 4 ||B||^2 (B = V^T E block) is reduced
on-device; the host sums the 8 partials (the "all-reduce") and divides
by B*N.

Measured: ~81-84 us HW exec per core (memory-bound; 22.5 MB of
compulsory HBM reads per core ~ 63 us at 358 GB/s, plus DMA-engine-0
instruction-fetch skew and the Tile drain/barrier tail).
"""

import os
from contextlib import ExitStack

import numpy as np

import concourse.bacc as bacc
import concourse.mybir as mybir
import concourse.tile as tile
from concourse.bass_utils import run_bass_kernel_spmd

B, F, T, D, S = 8, 256, 512, 40, 4
N = F * T              # rows per core (131072)
SD = S + D             # 44 combined features
P = 128                # partitions / chunk rows
N_CHUNKS = N // P      # 1024 matmul chunks per core
N_CORES = 8

# matmul dtype knob: float32 (exact, PE 4 cyc/row) or float16 (PE 1 cyc/row)
MM_DT_NAME = os.environ.get("KERNEL_MM_DT", "float16")
COL_TILE = os.environ.get("KERNEL_COL_TILE", "0") == "1"
# Keep partitions {0-3} and {124-127} free of DMA data.  DMA engine 0 also
# carries the kernel's instruction-fetch traffic (Q_XIV refills) and was the
# completion-semaphore straggler gating every block; engines 0/15 serve
# partitions {0-3,32-35} / {92-95,124-127}, so this halves their data load.
SKIP_P0 = os.environ.get("KERNEL_SKIP_P0", "0") == "1"

# block schedule: (chunks, data_partitions).  Small blocks at the start for
# fast pipeline fill.  With SKIP_P0, steady blocks put 120 rows per chunk on
# partitions [4:124) (strips zero-padded for the 128-row contraction) and a
# final 512-row block uses all 128 partitions.
if SKIP_P0:
    DP = 120
    BLOCK_SCHEDULE = [(16, DP), (16, DP), (32, DP)] + [(64, DP)] * 16 + [(4, P)]
else:
    DP = P
    BLOCK_SCHEDULE = [(16, P), (16, P), (32, P)] + [(64, P)] * 15
assert sum(u * dp for u, dp in BLOCK_SCHEDULE) == N

_nc_cache = {}


def _build_nc(key):
    mm_dt_name, col_tile = key
    mm_dt = getattr(mybir.dt, mm_dt_name)
    f32 = mybir.dt.float32
    cast = mm_dt != f32

    nc = bacc.Bacc("TRN2", target_bir_lowering=False, debug=False,
                   num_swdge_queues=int(os.environ.get("KERNEL_SWDGE_Q", "1")))
    E = nc.dram_tensor("embeddings", (N, D), f32, kind="ExternalInput")
    V = nc.dram_tensor("assignments", (N, S), f32, kind="ExternalInput")
    OUT = nc.dram_tensor("partial", (1, 1), f32, kind="ExternalOutput")

    with tile.TileContext(nc) as tc, ExitStack() as ctx:
        io_pool = ctx.enter_context(tc.tile_pool(name="io", bufs=4))
        w_pool = ctx.enter_context(tc.tile_pool(name="w", bufs=4))
        psum_pool = ctx.enter_context(tc.tile_pool(name="ps", bufs=1, space="PSUM"))
        # even chunks accumulate into partitions [0:SD] (PE col groups 0-1),
        # odd chunks into [64:64+SD] (col groups 2-3)
        g_ps = psum_pool.tile([64 + SD if col_tile else SD, SD], f32, tag="g")

        chunk = 0          # global chunk counter
        r0 = 0
        nblocks = len(BLOCK_SCHEDULE)
        for blk, (ub, dp) in enumerate(BLOCK_SCHEDULE):
            rows = dp * ub
            p0 = P - dp - 4 if dp < P else 0    # 4 for dp=120, 0 for dp=128
            e_ap = E[r0:r0 + rows, :].rearrange("(p u) d -> p (u d)", p=dp)
            v_ap = V[r0:r0 + rows, :].rearrange("(p u) s -> p (u s)", p=dp)
            r0 += rows
            kp = p0 + dp                # matmul contraction partitions
            # The first (small) pipeline-fill blocks go through HWDGE in
            # fp32 — no SWDGE Q7 bootstrap latency, ~2.5us faster first
            # data; the cast to fp16 happens in the interleave copies.
            # Steady blocks use SWDGE cast-DMAs (fp16 SBUF writes are
            # cheaper on the DMA engines).
            hw_start = cast and blk < 3
            io_dt = f32 if (not cast or hw_start) else mm_dt
            e_t = io_pool.tile([P, ub * D], io_dt, tag="e32" if hw_start else "e")
            v_t = io_pool.tile([P, ub * S], io_dt, tag="v32" if hw_start else "v")
            if p0:
                # compute ops need 32-aligned partition bases, so the zero
                # strip sits at [0:p0) and gets zeroed here (base 0 is
                # legal); the copies below propagate it into w_t
                nc.vector.memset(e_t[0:p0, :], 0.0)
                nc.vector.memset(v_t[0:p0, :], 0.0)
            if cast and not hw_start:
                nc.gpsimd.dma_start(out=e_t[p0:kp, :], in_=e_ap)
                nc.gpsimd.dma_start(out=v_t[p0:kp, :], in_=v_ap)
            else:
                nc.sync.dma_start(out=e_t[p0:kp, :], in_=e_ap)
                nc.sync.dma_start(out=v_t[p0:kp, :], in_=v_ap)

            # Interleave into per-chunk [V_u | E_u] blocks of 44 columns.
            w_t = w_pool.tile([P, ub * SD], mm_dt, tag="w")
            w3 = w_t[:].rearrange("p (u c) -> p u c", c=SD)
            nc.vector.tensor_copy(
                w3[0:kp, :, S:SD],
                e_t[0:kp, :].rearrange("p (u d) -> p u d", d=D),
            )
            nc.scalar.copy(
                w3[0:kp, :, 0:S],
                v_t[0:kp, :].rearrange("p (u s) -> p u s", s=S),
            )

            last_blk = blk == nblocks - 1
            for u in range(ub):
                wu = w_t[0:kp, u * SD:(u + 1) * SD]
                if col_tile:
                    half = chunk % 2
                    out_ap = g_ps[64 * half:64 * half + SD, :]
                    nc.tensor.matmul(
                        out_ap, wu, wu,
                        start=(chunk < 2),
                        stop=(last_blk and u >= ub - 2),
                        tile_position=(0, 64 * half),
                        skip_group_check=True,
                    )
                else:
                    nc.tensor.matmul(
                        g_ps[:], wu, wu,
                        start=(chunk == 0),
                        stop=(last_blk and u == ub - 1),
                    )
                chunk += 1

        # Epilogue: partial = sum(G^2) - 4 * sum(B^2), B = G[0:S, S:SD]
        ep = ctx.enter_context(tc.tile_pool(name="ep", bufs=1))
        g_sb = ep.tile([SD, SD], f32, tag="gsb")
        if col_tile:
            # DVE lanes can't read across partition bases, so shift the odd
            # half (partitions 64:108) down with a tiny SBUF->SBUF HWDGE DMA
            # and add the halves.
            o_sb = ep.tile([64 + SD, SD], f32, tag="osb")
            nc.vector.tensor_copy(o_sb[64:64 + SD, :], g_ps[64:64 + SD, :])
            shifted = ep.tile([SD, SD], f32, tag="sh")
            nc.sync.dma_start(out=shifted[:], in_=o_sb[64:64 + SD, :])
            nc.vector.tensor_add(g_sb[:], g_ps[0:SD, :], shifted[:])
        else:
            nc.vector.tensor_copy(g_sb[:], g_ps[0:SD, :])
        g2 = ep.tile([SD, SD], f32, tag="g2")
        nc.vector.tensor_mul(g2[:], g_sb[:], g_sb[:])
        colsum = ep.tile([SD, 1], f32, tag="cs")
        nc.vector.reduce_sum(colsum[:], g2[:], axis=mybir.AxisListType.X)
        bcol = ep.tile([S, 1], f32, tag="bc")
        nc.vector.reduce_sum(bcol[:], g2[0:S, S:SD], axis=mybir.AxisListType.X)
        bneg = ep.tile([S, 1], f32, tag="bn")
        nc.vector.tensor_scalar_mul(bneg[:], bcol[:], -4.0)
        ones = ep.tile([SD, 1], f32, tag="on")
        nc.vector.memset(ones[:], 1.0)
        s_ps = psum_pool.tile([1, 1], f32, tag="s")
        nc.tensor.matmul(s_ps[:], colsum[:], ones[:], start=True, stop=False)
        nc.tensor.matmul(s_ps[:], bneg[:], ones[0:S, :], start=False, stop=True)
        res = ep.tile([1, 1], f32, tag="r")
        nc.vector.tensor_copy(res[:], s_ps[:])
        nc.sync.dma_start(out=OUT[:, :], in_=res[:])

    nc.finalize()
    return nc


def _get_nc():
    key = (MM_DT_NAME, COL_TILE)
    if key not in _nc_cache:
        _nc_cache[key] = _build_nc(key)
    return _nc_cache[key]


def _run(embeddings: np.ndarray, assignments: np.ndarray, trace: bool = False):
    nc = _get_nc()
    in_maps = []
    for i in range(N_CORES):
        in_maps.append({
            "embeddings": np.ascontiguousarray(
                embeddings[i].reshape(N, D).astype(np.float32, copy=False)),
            "assignments": np.ascontiguousarray(
                assignments[i].reshape(N, S).astype(np.float32, copy=False)),
        })
    try:
        res = run_bass_kernel_spmd(
            nc, in_maps, core_ids=list(range(N_CORES)), trace=trace
        )
    except Exception:
        # transient NRT/device hiccups (e.g. NRT_EXEC_UNIT_UNRECOVERABLE)
        # have been observed to succeed on retry
        res = run_bass_kernel_spmd(
            nc, in_maps, core_ids=list(range(N_CORES)), trace=trace
        )
    partials = [float(r["partial"][0, 0]) for r in res.results]
    total = np.float32(np.sum(np.asarray(partials, dtype=np.float64)) / (B * N))
    return np.asarray(total, dtype=np.float32), res


def kernel(embeddings: np.ndarray, assignments: np.ndarray) -> np.ndarray:
    out, _ = _run(embeddings, assignments, trace=False)
    return out



# revision 2
# speedup vs baseline: 1.1506x; 1.1506x over previous
"""DeepClusteringLoss Trainium2 kernel.

loss = (||V^T V||_F^2 - 2 ||V^T E||_F^2 + ||E^T E||_F^2) / (B*N)
summed over batch, with E = embeddings.reshape(B, N, D), V =
assignments.reshape(B, N, S), N = F*T.

Sharding: data-parallel over batch; one core per batch element; the host
sums the 8 per-core partials (the scalar "all-reduce") and divides by
B*N.

Per-core pipeline (measured ~73-76 us HW exec on clean runs; an
environmental SDMA-engine-15 slowdown intermittently adds ~9 us):
- GLOBAL partition map: partition p owns rows [p*1024, (p+1)*1024).
  Chunk c = column c of every partition = 128 rows.
- All of V (2 MB) is cast-DMAed (fp32 HBM -> fp16 SBUF, SWDGE) up front
  into a resident tile; E streams as 13 column-slice cast-DMAs into
  resident tiles (no buffer reuse -> every DMA enqueues immediately,
  per-partition contiguous segments up to 20 KB, minimal descriptor
  overhead, ~96% SDMA occupancy).  num_swdge_queues=2.
- Interleave copies (DVE for E, ACT for V) build chunk-PAIR operands
  [V_2q | E_2q | pad20 | V_2q+1 | E_2q+1] (128 x 108 fp16): ONE matmul
  per two chunks -> 514 PE instructions instead of 2050 (the PE stream
  is 90% of instruction-fetch bytes, which ride DMA engine 0, the
  completion straggler).  Even/odd Grams accumulate at PSUM partition
  bases 0/64; pad and cross-term cells are never read.
- Epilogue just dumps the raw 108x108 PSUM accumulator; the host adds
  the even/odd diagonal blocks and reduces to the scalar partial in
  float64 (exact).
"""

import os
from contextlib import ExitStack

import numpy as np

import concourse.bacc as bacc
import concourse.mybir as mybir
import concourse.tile as tile
from concourse.bass_utils import run_bass_kernel_spmd

B, F, T, D, S = 8, 256, 512, 40, 4
N = F * T              # rows per core (131072)
SD = S + D             # 44 combined features
PW = 108               # paired-chunk width: 44 | 20 pad | 44
P = 128                # partitions
U = N // P             # rows per partition in the global map (1024)
N_CORES = 8

MM_DT_NAME = os.environ.get("KERNEL_MM_DT", "float16")
SWDGE_Q = int(os.environ.get("KERNEL_SWDGE_Q", "2"))
PAIRED = os.environ.get("KERNEL_PAIRED", "1") == "1"

# chunks per E-slice (all even).  First slice modest so the PE pipeline
# starts early; the tail tapers so the last transfer is tiny.
SLICES = [64] + [128] * 6 + [64, 48, 32, 24, 16, 8]
assert sum(SLICES) == U
assert all(ub % 2 == 0 for ub in SLICES)

_nc_cache = {}


def _build_nc(key):
    (mm_dt_name, paired) = key
    mm_dt = getattr(mybir.dt, mm_dt_name)
    f32 = mybir.dt.float32

    nc = bacc.Bacc("TRN2", target_bir_lowering=False, debug=False,
                   num_swdge_queues=SWDGE_Q)
    E = nc.dram_tensor("embeddings", (N, D), f32, kind="ExternalInput")
    V = nc.dram_tensor("assignments", (N, S), f32, kind="ExternalInput")
    OUT = nc.dram_tensor("partial", (PW, PW), f32, kind="ExternalOutput")

    # global-map DRAM views: partition p <- rows [p*U, (p+1)*U)
    e_g = E[:, :].rearrange("(p u) d -> p (u d)", p=P)   # [128, U*D]
    v_g = V[:, :].rearrange("(p u) s -> p (u s)", p=P)   # [128, U*S]

    with tile.TileContext(nc) as tc, ExitStack() as ctx:
        res_pool = ctx.enter_context(tc.tile_pool(name="res", bufs=1))
        w_pool = ctx.enter_context(tc.tile_pool(name="w", bufs=3))
        psum_pool = ctx.enter_context(tc.tile_pool(name="ps", bufs=1, space="PSUM"))
        gw = PW if paired else SD
        g_ps = psum_pool.tile([gw, gw], f32, tag="g")

        # V up front: one 2 MB cast-DMA into a resident fp16 tile.
        v_all = res_pool.tile([P, U * S], mm_dt, tag="v")
        nc.gpsimd.dma_start(out=v_all[:], in_=v_g)
        v3 = v_all[:].rearrange("p (u s) -> p u s", s=S)

        # E slices: resident fp16 tiles, one cast-DMA each.
        e_tiles = []
        c0 = 0
        for k, ub in enumerate(SLICES):
            e_t = res_pool.tile([P, ub * D], mm_dt, tag=f"e{k}")
            nc.gpsimd.dma_start(
                out=e_t[:], in_=e_g[:, c0 * D:(c0 + ub) * D])
            e_tiles.append((e_t, c0, ub))
            c0 += ub

        pair = 0
        chunk = 0
        n_pairs = N // (2 * P)
        for k, (e_t, c0, ub) in enumerate(e_tiles):
            last = k == len(e_tiles) - 1
            if paired:
                nq = ub // 2
                w_t = w_pool.tile([P, nq * PW], mm_dt, tag="w")
                w4 = w_t[:].rearrange("p (q c) -> p q c", c=PW)
                e2 = e_t[:].rearrange("p (q r) -> p q r", r=2 * D)
                v2 = v_all[:, c0 * S:(c0 + ub) * S].rearrange(
                    "p (q r) -> p q r", r=2 * S)
                nc.vector.tensor_copy(w4[:, :, S:SD], e2[:, :, 0:D])
                nc.vector.tensor_copy(w4[:, :, 64 + S:64 + SD], e2[:, :, D:2 * D])
                nc.scalar.copy(w4[:, :, 0:S], v2[:, :, 0:S])
                nc.scalar.copy(w4[:, :, 64:64 + S], v2[:, :, S:2 * S])
                for q in range(nq):
                    wq = w_t[:, q * PW:(q + 1) * PW]
                    nc.tensor.matmul(
                        g_ps[:], wq, wq,
                        start=(pair == 0),
                        stop=(last and q == nq - 1),
                    )
                    pair += 1
            else:
                w_t = w_pool.tile([P, ub * SD], mm_dt, tag="w")
                w3 = w_t[:].rearrange("p (u c) -> p u c", c=SD)
                nc.vector.tensor_copy(
                    w3[:, :, S:SD], e_t[:].rearrange("p (u d) -> p u d", d=D))
                nc.scalar.copy(w3[:, :, 0:S], v3[:, c0:c0 + ub, :])
                for u in range(ub):
                    wu = w_t[:, u * SD:(u + 1) * SD]
                    nc.tensor.matmul(
                        g_ps[:], wu, wu,
                        start=(chunk == 0),
                        stop=(last and u == ub - 1),
                    )
                    chunk += 1

        # Epilogue: dump the raw PSUM accumulator; the host adds the
        # even/odd diagonal Gram blocks and reduces to the scalar partial
        # (exact, in float64) alongside the cross-core sum.
        ep = ctx.enter_context(tc.tile_pool(name="ep", bufs=1))
        g_sb = ep.tile([gw, gw], f32, tag="gsb")
        nc.vector.tensor_copy(g_sb[:], g_ps[:])
        nc.sync.dma_start(out=OUT[0:gw, 0:gw], in_=g_sb[:])

    nc.finalize()
    return nc


def _get_nc():
    key = (MM_DT_NAME, PAIRED)
    if key not in _nc_cache:
        _nc_cache[key] = _build_nc(key)
    return _nc_cache[key]


def _run(embeddings: np.ndarray, assignments: np.ndarray, trace: bool = False):
    nc = _get_nc()
    in_maps = []
    for i in range(N_CORES):
        in_maps.append({
            "embeddings": np.ascontiguousarray(
                embeddings[i].reshape(N, D).astype(np.float32, copy=False)),
            "assignments": np.ascontiguousarray(
                assignments[i].reshape(N, S).astype(np.float32, copy=False)),
        })
    try:
        res = run_bass_kernel_spmd(
            nc, in_maps, core_ids=list(range(N_CORES)), trace=trace
        )
    except Exception:
        res = run_bass_kernel_spmd(
            nc, in_maps, core_ids=list(range(N_CORES)), trace=trace
        )
    partials = []
    for r in res.results:
        gp = np.asarray(r["partial"], dtype=np.float64)
        if PAIRED:
            G = gp[0:SD, 0:SD] + gp[64:64 + SD, 64:64 + SD]
        else:
            G = gp[0:SD, 0:SD]
        bm = G[0:S, S:SD]
        partials.append(np.sum(G * G) - 4.0 * np.sum(bm * bm))
    total = np.float32(np.sum(np.asarray(partials, dtype=np.float64)) / (B * N))
    return np.asarray(total, dtype=np.float32), res


def kernel(embeddings: np.ndarray, assignments: np.ndarray) -> np.ndarray:
    out, _ = _run(embeddings, assignments, trace=False)
    return out


# revision 3
# speedup vs baseline: 1.1609x; 1.0090x over previous
"""DeepClusteringLoss Trainium2 kernel.

loss = (||V^T V||_F^2 - 2 ||V^T E||_F^2 + ||E^T E||_F^2) / (B*N)
summed over batch, with E = embeddings.reshape(B, N, D), V =
assignments.reshape(B, N, S), N = F*T.

Sharding: data-parallel over batch; one core per batch element; the host
sums the 8 per-core partials (the scalar "all-reduce") and divides by
B*N.

Per-core pipeline (measured ~73-76 us HW exec on clean runs; an
environmental SDMA-engine-15 slowdown intermittently adds ~9 us):
- GLOBAL partition map: partition p owns rows [p*1024, (p+1)*1024).
  Chunk c = column c of every partition = 128 rows.
- All of V (2 MB) is cast-DMAed (fp32 HBM -> fp16 SBUF, SWDGE) up front
  into a resident tile; E streams as 13 column-slice cast-DMAs into
  resident tiles (no buffer reuse -> every DMA enqueues immediately,
  per-partition contiguous segments up to 20 KB, minimal descriptor
  overhead, ~96% SDMA occupancy).  num_swdge_queues=2.
- Interleave copies (DVE for E, ACT for V) build chunk-PAIR operands
  [V_2q | E_2q | pad20 | V_2q+1 | E_2q+1] (128 x 108 fp16): ONE matmul
  per two chunks -> 514 PE instructions instead of 2050 (the PE stream
  is 90% of instruction-fetch bytes, which ride DMA engine 0, the
  completion straggler).  Even/odd Grams accumulate at PSUM partition
  bases 0/64; pad and cross-term cells are never read.
- Epilogue dumps only the two 44x44 diagonal Gram blocks of the PSUM
  accumulator, each on its own HWDGE ring (SP/ACT) so their descriptor
  generations run in parallel; the host adds the blocks and reduces to
  the scalar partial in float64 (exact).
"""

import os
from contextlib import ExitStack

import numpy as np

import concourse.bacc as bacc
import concourse.mybir as mybir
import concourse.tile as tile
from concourse.bass_utils import run_bass_kernel_spmd

B, F, T, D, S = 8, 256, 512, 40, 4
N = F * T              # rows per core (131072)
SD = S + D             # 44 combined features
PW = 108               # paired-chunk width: 44 | 20 pad | 44
P = 128                # partitions
U = N // P             # rows per partition in the global map (1024)
N_CORES = 8

MM_DT_NAME = os.environ.get("KERNEL_MM_DT", "float16")
SWDGE_Q = int(os.environ.get("KERNEL_SWDGE_Q", "2"))
PAIRED = os.environ.get("KERNEL_PAIRED", "1") == "1"

# chunks per E-slice (all even).  First slice modest so the PE pipeline
# starts early; the tail tapers so the last transfer is tiny.
SLICES = [64] + [128] * 6 + [64, 48, 32, 24, 16, 8]
assert sum(SLICES) == U
assert all(ub % 2 == 0 for ub in SLICES)

_nc_cache = {}


def _build_nc(key):
    (mm_dt_name, paired) = key
    mm_dt = getattr(mybir.dt, mm_dt_name)
    f32 = mybir.dt.float32

    nc = bacc.Bacc("TRN2", target_bir_lowering=False, debug=False,
                   num_swdge_queues=SWDGE_Q)
    E = nc.dram_tensor("embeddings", (N, D), f32, kind="ExternalInput")
    V = nc.dram_tensor("assignments", (N, S), f32, kind="ExternalInput")
    OUT = nc.dram_tensor("partial", (PW, PW), f32, kind="ExternalOutput")

    # global-map DRAM views: partition p <- rows [p*U, (p+1)*U)
    e_g = E[:, :].rearrange("(p u) d -> p (u d)", p=P)   # [128, U*D]
    v_g = V[:, :].rearrange("(p u) s -> p (u s)", p=P)   # [128, U*S]

    with tile.TileContext(nc) as tc, ExitStack() as ctx:
        res_pool = ctx.enter_context(tc.tile_pool(name="res", bufs=1))
        w_pool = ctx.enter_context(tc.tile_pool(name="w", bufs=3))
        psum_pool = ctx.enter_context(tc.tile_pool(name="ps", bufs=1, space="PSUM"))
        gw = PW if paired else SD
        g_ps = psum_pool.tile([gw, gw], f32, tag="g")

        # V up front: one 2 MB cast-DMA into a resident fp16 tile.
        v_all = res_pool.tile([P, U * S], mm_dt, tag="v")
        nc.gpsimd.dma_start(out=v_all[:], in_=v_g)
        v3 = v_all[:].rearrange("p (u s) -> p u s", s=S)

        # E slices: resident fp16 tiles, one cast-DMA each.
        e_tiles = []
        c0 = 0
        for k, ub in enumerate(SLICES):
            e_t = res_pool.tile([P, ub * D], mm_dt, tag=f"e{k}")
            nc.gpsimd.dma_start(
                out=e_t[:], in_=e_g[:, c0 * D:(c0 + ub) * D])
            e_tiles.append((e_t, c0, ub))
            c0 += ub

        pair = 0
        chunk = 0
        n_pairs = N // (2 * P)
        for k, (e_t, c0, ub) in enumerate(e_tiles):
            last = k == len(e_tiles) - 1
            if paired:
                nq = ub // 2
                w_t = w_pool.tile([P, nq * PW], mm_dt, tag="w")
                w4 = w_t[:].rearrange("p (q c) -> p q c", c=PW)
                e2 = e_t[:].rearrange("p (q r) -> p q r", r=2 * D)
                v2 = v_all[:, c0 * S:(c0 + ub) * S].rearrange(
                    "p (q r) -> p q r", r=2 * S)
                nc.vector.tensor_copy(w4[:, :, S:SD], e2[:, :, 0:D])
                nc.vector.tensor_copy(w4[:, :, 64 + S:64 + SD], e2[:, :, D:2 * D])
                nc.scalar.copy(w4[:, :, 0:S], v2[:, :, 0:S])
                nc.scalar.copy(w4[:, :, 64:64 + S], v2[:, :, S:2 * S])
                for q in range(nq):
                    wq = w_t[:, q * PW:(q + 1) * PW]
                    nc.tensor.matmul(
                        g_ps[:], wq, wq,
                        start=(pair == 0),
                        stop=(last and q == nq - 1),
                    )
                    pair += 1
            else:
                w_t = w_pool.tile([P, ub * SD], mm_dt, tag="w")
                w3 = w_t[:].rearrange("p (u c) -> p u c", c=SD)
                nc.vector.tensor_copy(
                    w3[:, :, S:SD], e_t[:].rearrange("p (u d) -> p u d", d=D))
                nc.scalar.copy(w3[:, :, 0:S], v3[:, c0:c0 + ub, :])
                for u in range(ub):
                    wu = w_t[:, u * SD:(u + 1) * SD]
                    nc.tensor.matmul(
                        g_ps[:], wu, wu,
                        start=(chunk == 0),
                        stop=(last and u == ub - 1),
                    )
                    chunk += 1

        # Epilogue: dump only the two 44x44 diagonal Gram blocks of the
        # PSUM accumulator, each on its own HWDGE ring (SP and ACT) so the
        # descriptor generation for the two OUT transfers runs in
        # parallel; the host adds the blocks and reduces to the scalar
        # partial (exact, in float64) alongside the cross-core sum.
        ep = ctx.enter_context(tc.tile_pool(name="ep", bufs=1))
        if paired:
            ge_sb = ep.tile([SD, SD], f32, tag="ge")
            go_sb = ep.tile([64 + SD, SD], f32, tag="go")
            nc.vector.tensor_copy(ge_sb[:], g_ps[0:SD, 0:SD])
            nc.vector.tensor_copy(
                go_sb[64:64 + SD, :], g_ps[64:64 + SD, 64:64 + SD])
            nc.sync.dma_start(out=OUT[0:SD, 0:SD], in_=ge_sb[:])
            nc.scalar.dma_start(
                out=OUT[64:64 + SD, 64:64 + SD], in_=go_sb[64:64 + SD, :])
        else:
            g_sb = ep.tile([gw, gw], f32, tag="gsb")
            nc.vector.tensor_copy(g_sb[:], g_ps[:])
            nc.sync.dma_start(out=OUT[0:gw, 0:gw], in_=g_sb[:])

    nc.finalize()
    return nc


def _get_nc():
    key = (MM_DT_NAME, PAIRED)
    if key not in _nc_cache:
        _nc_cache[key] = _build_nc(key)
    return _nc_cache[key]


def _run(embeddings: np.ndarray, assignments: np.ndarray, trace: bool = False):
    nc = _get_nc()
    in_maps = []
    for i in range(N_CORES):
        in_maps.append({
            "embeddings": np.ascontiguousarray(
                embeddings[i].reshape(N, D).astype(np.float32, copy=False)),
            "assignments": np.ascontiguousarray(
                assignments[i].reshape(N, S).astype(np.float32, copy=False)),
        })
    try:
        res = run_bass_kernel_spmd(
            nc, in_maps, core_ids=list(range(N_CORES)), trace=trace
        )
    except Exception:
        res = run_bass_kernel_spmd(
            nc, in_maps, core_ids=list(range(N_CORES)), trace=trace
        )
    partials = []
    for r in res.results:
        gp = np.asarray(r["partial"], dtype=np.float64)
        if PAIRED:
            G = gp[0:SD, 0:SD] + gp[64:64 + SD, 64:64 + SD]
        else:
            G = gp[0:SD, 0:SD]
        bm = G[0:S, S:SD]
        partials.append(np.sum(G * G) - 4.0 * np.sum(bm * bm))
    total = np.float32(np.sum(np.asarray(partials, dtype=np.float64)) / (B * N))
    return np.asarray(total, dtype=np.float32), res


def kernel(embeddings: np.ndarray, assignments: np.ndarray) -> np.ndarray:
    out, _ = _run(embeddings, assignments, trace=False)
    return out
